# revision 1
# baseline (speedup 1.0000x reference)
"""Trainium2 Bass kernel for nn_JointModalityAttention.

3-modality joint attention, B=8, N=512, D=512, 8 heads x 64.
Sharding: data-parallel over batch -- each of the 8 NeuronCores handles one
batch element; the weights are replicated. No collectives.

Dataflow (per core, fully transpose-free on device):
  - Host passes x transposed (xT[k, n]) and a q-projection copy with masked
    query rows zeroed (xqT).  Zeroed q rows -> dots == 0 for those queries ->
    exp(0) == 1 -> uniform attention, which is exactly what the reference's
    whole-row -1e9 masking produces.
  - Transposed projections give qT[c, n], kT[c, n] directly (lhsT = Wqkv
    column chunk, rhs = xT); the V projection runs natural (lhsT = xT chunk,
    rhs = Wv) giving V[m, d], stored head-interleaved with a ones column per
    head (V_ext[m, 65]).
  - dots^T[m, n] = kT_h.T @ qT_h  (contraction over the 64-dim head axis on
    partitions; no transposes needed).
  - exp on ScalarE straight out of PSUM, with the 1/sqrt(dh) scale folded in.
    No max-subtraction: dots are O(1) here, and softmax(x) == exp(x)/sum.
  - o^T[d, n] (+ denominator row) = V_ext.T @ expT; the ones column of V_ext
    makes row 64 the softmax denominator.
  - normalize with reciprocal + partition-broadcast + multiply into
    oT[c, n], which is exactly the lhsT layout the out-projection needs.
"""

import sys

if "/opt/trn_rl_repo" not in sys.path:
    sys.path.insert(0, "/opt/trn_rl_repo")

import numpy as np

import concourse.bass as bass  # noqa: F401  (import keeps bass registered)
import concourse.mybir as mybir
import concourse.tile as tile
from concourse import bacc, bass_utils

HEADS = 8
DH = 64
DI = HEADS * DH  # 512
B = 8
N = 512
D = 512
SCALE = DH ** -0.5
N_CORES = 8

F32 = mybir.dt.float32
DT_MM = mybir.dt.float32r  # matmul operand dtype (f32r == TF32: full PE rate at N>=256)


def tf32_round(a):
    """Round fp32 to the TF32-representable subset (10-bit mantissa, RNE)."""
    u = np.ascontiguousarray(a, np.float32).view(np.uint32).copy()
    lsb = (u >> 13) & 1
    u += 0x0FFF + lsb
    u &= 0xFFFFE000
    return u.view(np.float32)


def _emit_body(nc, tc, dio, pools, n_c):
    f32 = F32
    Exp = mybir.ActivationFunctionType.Exp
    p_w, p_xt, p_xqt, p_qkv, p_exp, p_den, p_oT, p_wo, p_ob, psA, psB = pools

    def mm(out, lhsT, rhs, start, stop):
        nc.tensor.matmul(out, lhsT, rhs, start=start, stop=stop)

    qT, kT, V, oT = {}, {}, {}, {}

    ones8 = p_den.tile([128, 8], f32, tag="ones8", name="ones8")
    nc.vector.memset(ones8[:, :], 1.0)

    # ---- projections ----
    for i in range(3):
        w_t = []
        for kt in range(4):
            w = p_w.tile([128, 1536], DT_MM, tag="w", name=f"w{i}_{kt}")
            nc.sync.dma_start(w[:, :], dio[f"Wqkv{i}"][kt * 128 : (kt + 1) * 128, :])
            w_t.append(w)
        xt_t = p_xt.tile([128, 4 * 512], DT_MM, tag="xt", name=f"xt{i}")
        xqt_t = p_xqt.tile([128, 4 * n_c], DT_MM, tag="xqt", name=f"xqt{i}")
        for kt in range(4):
            nc.sync.dma_start(
                xt_t[:, kt * 512 : (kt + 1) * 512],
                dio[f"xT{i}"][kt * 128 : (kt + 1) * 128, :],
            )
            nc.sync.dma_start(
                xqt_t[:, kt * n_c : (kt + 1) * n_c],
                dio[f"xqT{i}"][kt * 128 : (kt + 1) * 128, :],
            )
        qT[i] = p_qkv.tile([128, 4 * n_c], DT_MM, tag=f"qT{i}", name=f"qT{i}")
        kT[i] = p_qkv.tile([128, 4 * 512], DT_MM, tag=f"kT{i}", name=f"kT{i}")
        V[i] = p_qkv.tile([128, 4 * 520], DT_MM, tag=f"V{i}", name=f"V{i}")
        oT[i] = p_oT.tile([128, 4 * n_c], DT_MM, tag=f"oT{i}", name=f"oT{i}")

        # qT chunks (transposed projection, rhs = masked xqT)
        for cc in range(4):
            ps = psB.tile([128, 512], f32, tag="mm", name=f"psq{i}_{cc}")
            for kt in range(4):
                mm(
                    ps[:, 0:n_c],
                    w_t[kt][:, cc * 128 : (cc + 1) * 128],
                    xqt_t[:, kt * n_c : (kt + 1) * n_c],
                    kt == 0,
                    kt == 3,
                )
            nc.vector.tensor_copy(qT[i][:, cc * n_c : (cc + 1) * n_c], ps[:, 0:n_c])
        # kT chunks
        for cc in range(4):
            ps = psB.tile([128, 512], f32, tag="mm", name=f"psk{i}_{cc}")
            for kt in range(4):
                mm(
                    ps[:, :],
                    w_t[kt][:, 512 + cc * 128 : 512 + (cc + 1) * 128],
                    xt_t[:, kt * 512 : (kt + 1) * 512],
                    kt == 0,
                    kt == 3,
                )
            nc.vector.tensor_copy(kT[i][:, cc * 512 : (cc + 1) * 512], ps[:, :])
        # V chunks (natural layout over keys), head-interleaved with ones col
        for nch in range(4):
            ps = psB.tile([128, 512], f32, tag="mm", name=f"psv{i}_{nch}")
            for kt in range(4):
                mm(
                    ps[:, :],
                    xt_t[:, kt * 512 + nch * 128 : kt * 512 + nch * 128 + 128],
                    w_t[kt][:, 1024:1536],
                    kt == 0,
                    kt == 3,
                )
            dst = V[i][:, nch * 520 : (nch + 1) * 520].rearrange(
                "p (h x) -> p h x", x=65
            )
            nc.vector.tensor_copy(
                dst[:, :, 0:64], ps[:, :].rearrange("p (h x) -> p h x", x=64)
            )
            nc.vector.tensor_copy(
                dst[:, :, 64:65], ones8[:, :].rearrange("p (h x) -> p h x", x=1)
            )

    # ---- attention ----
    mjs = [(j, c) for j in range(3) for c in range(4)]
    for i in range(3):
        for h in range(8):
            po = (h % 2) * 64
            fh = (h // 2)
            qh = qT[i][po : po + 64, fh * n_c : fh * n_c + n_c]
            ps_o = psB.tile([128, 512], f32, tag="mm", name=f"pso{i}_{h}")
            for t in range(4):  # 12 m-chunks in 4 batches of 3
                ps_d = psA.tile([128, 3 * 512], f32, tag="dots", name=f"psd{i}_{h}_{t}")
                for u in range(3):
                    j, c = mjs[t * 3 + u]
                    kh = kT[j][
                        po : po + 64, fh * 512 + c * 128 : fh * 512 + c * 128 + 128
                    ]
                    mm(ps_d[:, u * 512 : u * 512 + n_c], kh, qh, True, True)
                ex = p_exp.tile([128, 3 * n_c], DT_MM, tag="exp", name=f"ex{i}_{h}_{t}")
                src = ps_d[:, :].rearrange("p (u x) -> p u x", x=512)[:, :, 0:n_c]
                nc.scalar.activation(
                    ex[:, :].rearrange("p (u x) -> p u x", x=n_c),
                    src,
                    Exp,
                    scale=SCALE,
                )
                for u in range(3):
                    j, c = mjs[t * 3 + u]
                    vh = V[j][:, c * 520 + h * 65 : c * 520 + h * 65 + 65]
                    mm(
                        ps_o[0:65, 0:n_c],
                        vh,
                        ex[:, u * n_c : (u + 1) * n_c],
                        t == 0 and u == 0,
                        t == 3 and u == 2,
                    )
            den_s = p_den.tile([1, 512], f32, tag="dens", name=f"ds{i}_{h}")
            nc.vector.reciprocal(den_s[0:1, 0:n_c], ps_o[64:65, 0:n_c])
            den_r = p_den.tile([64, 512], f32, tag="denr", name=f"dr{i}_{h}")
            nc.gpsimd.partition_broadcast(den_r[:, 0:n_c], den_s[0:1, 0:n_c])
            dst = oT[i][po : po + 64, fh * n_c : fh * n_c + n_c]
            nc.vector.tensor_mul(dst, ps_o[0:64, 0:n_c], den_r[:, 0:n_c])

    # ---- out-projection ----
    nch_list = [(s, min(128, n_c - s)) for s in range(0, n_c, 128)]
    for i in range(3):
        wo_t = p_wo.tile([128, 4 * 512], DT_MM, tag="wo", name=f"wo{i}")
        for kt in range(4):
            nc.sync.dma_start(
                wo_t[:, kt * 512 : (kt + 1) * 512],
                dio[f"Wout{i}"][kt * 128 : (kt + 1) * 128, :],
            )
        for s, nn in nch_list:
            ps = psB.tile([128, 512], f32, tag="mm", name=f"psf{i}_{s}")
            for ct in range(4):
                mm(
                    ps[0:nn, :],
                    oT[i][:, ct * n_c + s : ct * n_c + s + nn],
                    wo_t[:, ct * 512 : (ct + 1) * 512],
                    ct == 0,
                    ct == 3,
                )
            ob = p_ob.tile([128, 512], f32, tag="ob", name=f"ob{i}_{s}")
            nc.vector.tensor_copy(ob[0:nn, :], ps[0:nn, :])
            nc.sync.dma_start(dio[f"out{i}"][s : s + nn, :], ob[0:nn, :])


def build(n_c=512, reps=1):
    nc = bacc.Bacc("TRN2", target_bir_lowering=False, debug=False)
    dio = {}
    for i in range(3):
        dio[f"xT{i}"] = nc.dram_tensor(f"xT{i}", [D, N], DT_MM, kind="ExternalInput").ap()
        dio[f"xqT{i}"] = nc.dram_tensor(
            f"xqT{i}", [D, n_c], DT_MM, kind="ExternalInput"
        ).ap()
        dio[f"Wqkv{i}"] = nc.dram_tensor(
            f"Wqkv{i}", [D, 3 * DI], DT_MM, kind="ExternalInput"
        ).ap()
        dio[f"Wout{i}"] = nc.dram_tensor(
            f"Wout{i}", [DI, D], DT_MM, kind="ExternalInput"
        ).ap()
        dio[f"out{i}"] = nc.dram_tensor(
            f"out{i}", [n_c, D], F32, kind="ExternalOutput"
        ).ap()
    with tile.TileContext(nc) as tc:
        with (
            tc.tile_pool(name="wq", bufs=5) as p_w,
            tc.tile_pool(name="xt", bufs=2) as p_xt,
            tc.tile_pool(name="xqt", bufs=2) as p_xqt,
            tc.tile_pool(name="qkv", bufs=1) as p_qkv,
            tc.tile_pool(name="exp", bufs=2) as p_exp,
            tc.tile_pool(name="den", bufs=3) as p_den,
            tc.tile_pool(name="oT", bufs=1) as p_oT,
            tc.tile_pool(name="wo", bufs=1) as p_wo,
            tc.tile_pool(name="ob", bufs=2) as p_ob,
            tc.tile_pool(name="psA", bufs=2, space="PSUM") as psA,
            tc.tile_pool(name="psB", bufs=2, space="PSUM") as psB,
        ):
            pools = (p_w, p_xt, p_xqt, p_qkv, p_exp, p_den, p_oT, p_wo, p_ob, psA, psB)
            for _ in range(reps):
                _emit_body(nc, tc, dio, pools, n_c)
    nc.compile()
    return nc


_BUILD_CACHE = {}


def _get_built(n_c, reps):
    key = (n_c, reps)
    if key not in _BUILD_CACHE:
        _BUILD_CACHE[key] = build(n_c, reps)
    return _BUILD_CACHE[key]


def pick_n_c(inputs):
    """Smallest supported compacted-query count for these masks.

    Per (b, i) we need room for the unmasked queries plus one zero (dummy)
    row whose output serves every masked query of that (b, i).
    """
    need = 0
    for i in range(3):
        m = np.asarray(inputs[f"m{i}"]).astype(bool)
        for b in range(B):
            n_u = int(m[b].sum())
            need = max(need, n_u + (1 if n_u < N else 0))
    for cand in (320, 384, 448, 512):
        if need <= cand:
            return cand
    return 512


def make_in_maps(inputs, n_c=512):
    """Build per-core input dicts.  When n_c < N, compact the q-projection
    input to the unmasked query rows (plus zero padding; the first padding
    row doubles as the masked-row output)."""
    rnd = tf32_round if DT_MM == mybir.dt.float32r else (
        lambda a: np.ascontiguousarray(a, np.float32)
    )
    xs = [np.asarray(inputs[f"x{i}"], dtype=np.float32) for i in range(3)]
    ms = [np.asarray(inputs[f"m{i}"]).astype(bool) for i in range(3)]
    Wq = [rnd(np.asarray(inputs[f"Wqkv{i}"], np.float32)) for i in range(3)]
    Wo = [rnd(np.asarray(inputs[f"Wout{i}"], np.float32)) for i in range(3)]
    in_maps = []
    for b in range(B):
        m = {}
        for i in range(3):
            xb = xs[i][b]
            m[f"xT{i}"] = rnd(xb.T)
            if n_c == N:
                m[f"xqT{i}"] = rnd((xb * ms[i][b][:, None]).T)
            else:
                sel = np.flatnonzero(ms[i][b])
                xq = np.zeros((n_c, D), np.float32)
                xq[: len(sel)] = xb[sel]
                m[f"xqT{i}"] = rnd(xq.T)
            m[f"Wqkv{i}"] = Wq[i]
            m[f"Wout{i}"] = Wo[i]
        in_maps.append(m)
    return in_maps


def scatter_outputs(results, inputs, n_c):
    ms = [np.asarray(inputs[f"m{i}"]).astype(bool) for i in range(3)]
    outs = []
    for i in range(3):
        full = np.empty((B, N, D), np.float32)
        for b in range(B):
            comp = np.asarray(results[b][f"out{i}"], np.float32)
            if n_c == N:
                full[b] = comp
            else:
                sel = np.flatnonzero(ms[i][b])
                full[b][sel] = comp[: len(sel)]
                if len(sel) < N:
                    full[b][~ms[i][b]] = comp[len(sel)]
        outs.append(full)
    return outs


def kernel(**inputs):
    n_c = pick_n_c(inputs)
    in_maps = make_in_maps(inputs, n_c)
    nc = _get_built(n_c=n_c, reps=1)
    res = bass_utils.run_bass_kernel_spmd(nc, in_maps, core_ids=list(range(N_CORES)))
    return tuple(scatter_outputs(res.results, inputs, n_c))

